# revision 21
# baseline (speedup 1.0000x reference)
"""MoE gate (LLaDA2) routing kernel for 8 Trainium2 NeuronCores.

Strategy: token-parallel over 8 cores (2048 tokens/core). Router GEMM as
fp16 main term (xhi@whi, exact at FP22 internal precision) plus BOTH fp16
residual cross-terms (xhi@wlo + xlo@whi) computed in a single fp8-e5m2
DoubleRow matmul per contraction chunk. The e5m2 operands carry compensating
power-of-two scales (xhi*2^-7 (x) wlo*2^7, xlo*2^6 (x) whi*2^-6) so the DR
products are unscaled and accumulate straight into the main fp32 PSUM —
2 matmuls/chunk instead of 3, ~1.4x less PE streaming than the 3-term split,
with routing accuracy within a few flipped indices of the fp32 reference.
Routing epilogue: fused scalar_tensor_tensor selection + gpsimd offload.
"""
import sys
for p in ("/opt/trn_rl_repo", "/root/.axon_site/_ro/trn_rl_repo"):
    if p not in sys.path:
        sys.path.append(p)

import numpy as np

T, H, E = 16384, 4096, 256
NCORES = 8
TPC = T // NCORES          # tokens per core: 2048
NTILES = TPC // 128        # 16 row tiles
KCH = H // 128             # 32 contraction chunks
WSPLIT = 8                 # w DMA split for early start
G = 8                      # expert groups
GS = E // G                # 32 experts/group
K = 8                      # top-k
NEG = -1.0e4

_cache = {}


def _build():
    import concourse.bacc as bacc
    import concourse.bass as bass
    import concourse.mybir as mybir
    from concourse import tile

    dt = mybir.dt
    Alu = mybir.AluOpType
    Act = mybir.ActivationFunctionType
    Ax = mybir.AxisListType
    DR = mybir.MatmulPerfMode.DoubleRow

    nc = bacc.Bacc("TRN2", target_bir_lowering=False, debug=False,
                   num_devices=NCORES)

    xhi_d = nc.dram_tensor("xhi", [NTILES, 128, KCH, 128], dt.float16, kind="ExternalInput")
    xdr_d = nc.dram_tensor("xdr", [NTILES, 128, KCH, 2, 128], dt.float8e5, kind="ExternalInput")
    whi_d = nc.dram_tensor("whi", [WSPLIT, 128, KCH // WSPLIT, E], dt.float16, kind="ExternalInput")
    wdr_d = nc.dram_tensor("wdr", [WSPLIT, 128, KCH // WSPLIT, 2, E], dt.float8e5, kind="ExternalInput")
    btab_d = nc.dram_tensor("btab", [128, E], dt.float32, kind="ExternalInput")
    w_out = nc.dram_tensor("w_out", [TPC, K], dt.float32, kind="ExternalOutput")
    i_out = nc.dram_tensor("i_out", [TPC, K], dt.uint32, kind="ExternalOutput")

    KPW = KCH // WSPLIT  # k-chunks per w split

    def bc_mid(ap8, n=8):
        # [128, m] -> [128, n(bcast), m]
        return bass.AP(ap8.tensor, ap8.offset, [list(ap8.ap[0]), [0, n], list(ap8.ap[1])])

    def pair_ap(t, elem_off, m):
        # slice of flat SBUF tile -> [128, 2, m] AP (pair-slot stride m)
        a = t[:]
        return bass.AP(a.tensor, a.offset + elem_off, [list(a.ap[0]), [m, 2], [1, m]])

    with tile.TileContext(nc) as tc:
        with (
            tc.tile_pool(name="wpool", bufs=1) as wpool,
            tc.tile_pool(name="xpool", bufs=4) as xpool,
            tc.tile_pool(name="ppool", bufs=4, space="PSUM") as ppool,
            tc.tile_pool(name="spool", bufs=3) as spool,
            tc.tile_pool(name="tpool", bufs=3) as tpool,
            tc.tile_pool(name="opool", bufs=1) as opool,
        ):
            whis, wdrs = [], []
            for s in range(WSPLIT):
                whi_t = wpool.tile([128, KPW * E], dt.float16, tag=f"whi{s}")
                wdr_t = wpool.tile([128, KPW * 2 * E], dt.float8e5, tag=f"wdr{s}")
                whis.append(whi_t)
                wdrs.append(wdr_t)
            nc.sync.dma_start(whis[0][:], whi_d[0].rearrange("p k e -> p (k e)"))
            nc.sync.dma_start(wdrs[0][:], wdr_d[0].rearrange("p k u e -> p (k u e)"))
            btab = wpool.tile([128, E], dt.float32, tag="btab")
            nc.sync.dma_start(btab[:], btab_d[:])

            out_w = opool.tile([128, NTILES * K], dt.float32, tag="ow")
            out_i = opool.tile([128, NTILES * K], dt.uint32, tag="oi")

            XG = 2                    # x k-group split for DMA granularity
            KPX = KCH // XG
            for i in range(NTILES):
                xhgs, xdgs = [], []
                for g in range(XG):
                    xhg = xpool.tile([128, KPX * 128], dt.float16, tag=f"xh{g}")
                    nc.sync.dma_start(xhg[:], xhi_d[i, :, g * KPX:(g + 1) * KPX, :]
                                      .rearrange("p k t -> p (k t)"))
                    xdg = xpool.tile([128, KPX * 2 * 128], dt.float8e5, tag=f"xd{g}")
                    nc.sync.dma_start(xdg[:], xdr_d[i, :, g * KPX:(g + 1) * KPX, :, :]
                                      .rearrange("p k u t -> p (k u t)"))
                    xhgs.append(xhg)
                    xdgs.append(xdg)

                if i in (0, 1):
                    for s in ((1, 2) if i == 0 else (3,)):
                        nc.sync.dma_start(whis[s][:], whi_d[s].rearrange("p k e -> p (k e)"))
                        nc.sync.dma_start(wdrs[s][:], wdr_d[s].rearrange("p k u e -> p (k u e)"))

                psum = ppool.tile([128, E], dt.float32, tag="ps")
                for k in range(KCH):
                    xh = xhgs[k // KPX][:, (k % KPX) * 128:(k % KPX + 1) * 128]
                    wh = whis[k // KPW][:, (k % KPW) * E:(k % KPW + 1) * E]
                    nc.tensor.matmul(psum[:], lhsT=xh, rhs=wh,
                                     start=(k == 0), stop=False)
                    xd3 = pair_ap(xdgs[k // KPX], (k % KPX) * 2 * 128, 128)
                    wd3 = pair_ap(wdrs[k // KPW], (k % KPW) * 2 * E, E)
                    nc.tensor.matmul(psum[:], lhsT=xd3, rhs=wd3,
                                     start=False, stop=(k == KCH - 1), perf_mode=DR)

                # --- routing epilogue (DVE-centric, gpsimd offload) ---
                scores = spool.tile([128, E], dt.float32, tag="scores")
                nc.scalar.activation(scores[:], psum[:], Act.Sigmoid)

                # sr = scores + bias (selection scores)
                sr = spool.tile([128, E], dt.float32, tag="sr")
                nc.vector.tensor_tensor(sr[:], scores[:], btab[:], Alu.add)
                sr3 = sr[:].rearrange("p (g e) -> p g e", g=G)

                # group top-2 sum via top1 / match_replace / top2
                top1 = tpool.tile([128, G], dt.float32, tag="top1")
                nc.vector.tensor_reduce(top1[:], sr3, axis=Ax.X, op=Alu.max)
                mr2 = spool.tile([128, E], dt.float32, tag="mr2")
                nc.vector.match_replace(mr2[:], in_to_replace=top1[:], in_values=sr[:], imm_value=NEG)
                top2 = tpool.tile([128, G], dt.float32, tag="top2")
                nc.vector.tensor_reduce(top2[:], mr2[:].rearrange("p (g e) -> p g e", g=G),
                                        axis=Ax.X, op=Alu.max)
                gs_t = tpool.tile([128, G], dt.float32, tag="gs")
                nc.vector.tensor_tensor(gs_t[:], top1[:], top2[:], Alu.add)

                # keep top-4 groups: threshold at 4th largest group score
                g8 = tpool.tile([128, 8], dt.float32, tag="g8")
                nc.vector.max(out=g8[:], in_=gs_t[:])
                inv = tpool.tile([128, G], dt.float32, tag="inv")
                nc.vector.tensor_scalar(inv[:], gs_t[:], g8[:, 3:4], -NEG, op0=Alu.is_lt, op1=Alu.mult)
                # mask: sr -= inv (0 for kept groups, 1e4 for dropped)
                nc.vector.tensor_tensor(sr3, sr3, inv[:].to_broadcast([128, G, GS]), Alu.subtract)

                # top-8 selection on masked sr
                vals8 = tpool.tile([128, K], dt.float32, tag="vals8")
                nc.vector.max(out=vals8[:], in_=sr[:])
                idx8 = tpool.tile([128, K], dt.uint32, tag="idx8")
                nc.vector.max_index(out=idx8[:], in_max=vals8[:], in_values=sr[:])

                # scores at selected positions: (sr >= t8) * scores, one fused op
                selm = spool.tile([128, E], dt.float32, tag="selm")
                nc.vector.scalar_tensor_tensor(selm[:], sr[:], vals8[:, 7:8], scores[:],
                                               op0=Alu.is_ge, op1=Alu.mult)
                svals8 = tpool.tile([128, K], dt.float32, tag="svals8")
                nc.vector.max(out=svals8[:], in_=selm[:])
                sidx8 = tpool.tile([128, K], dt.uint32, tag="sidx8")
                nc.vector.max_index(out=sidx8[:], in_max=svals8[:], in_values=selm[:])

                # reorder svals8 (score-sorted) into idx8 (sr-sorted) slots
                idx8f = tpool.tile([128, K], dt.float32, tag="idx8f")
                nc.gpsimd.tensor_copy(idx8f[:], idx8[:])
                sidx8f = tpool.tile([128, K], dt.float32, tag="sidx8f")
                nc.gpsimd.tensor_copy(sidx8f[:], sidx8[:])
                eq = tpool.tile([128, K * K], dt.float32, tag="eq")
                eq3 = eq[:].rearrange("p (k j) -> p k j", k=K)
                nc.vector.tensor_tensor(eq3, idx8f[:].to_broadcast([128, K, K]), bc_mid(sidx8f[:]), Alu.is_equal)
                prod = tpool.tile([128, K * K], dt.float32, tag="prod")
                prod3 = prod[:].rearrange("p (k j) -> p k j", k=K)
                nc.vector.tensor_tensor(prod3, eq3, bc_mid(svals8[:]), Alu.mult)
                w8 = tpool.tile([128, K], dt.float32, tag="w8")
                nc.vector.tensor_reduce(w8[:], prod3, axis=Ax.X, op=Alu.add)

                sum8 = tpool.tile([128, 1], dt.float32, tag="sum8")
                nc.vector.tensor_reduce(sum8[:], w8[:], axis=Ax.X, op=Alu.add)
                rec = tpool.tile([128, 1], dt.float32, tag="rec")
                nc.vector.reciprocal(rec[:], sum8[:])

                nc.vector.tensor_scalar(out_w[:, i * K:(i + 1) * K], w8[:], rec[:, 0:1], 2.5,
                                        op0=Alu.mult, op1=Alu.mult)
                nc.gpsimd.tensor_copy(out_i[:, i * K:(i + 1) * K], idx8[:])

            nc.sync.dma_start(w_out[:].rearrange("(i p) k -> p i k", p=128),
                              out_w[:].rearrange("p (i k) -> p i k", i=NTILES))
            nc.sync.dma_start(i_out[:].rearrange("(i p) k -> p i k", p=128),
                              out_i[:].rearrange("p (i k) -> p i k", i=NTILES))

    nc.compile()
    return nc


def _prep(hidden_states, weight, expert_bias):
    import ml_dtypes
    e5 = ml_dtypes.float8_e5m2
    x = np.ascontiguousarray(hidden_states, dtype=np.float32)
    w = np.ascontiguousarray(weight, dtype=np.float32)
    whi = w.astype(np.float16)
    wlo = w - whi.astype(np.float32)
    # DR pair slots (scale-compensated e5m2): slot0 = wlo*2^7, slot1 = whi*2^-6
    wl8 = (wlo * 2.0**7).astype(e5)
    wh8 = (whi.astype(np.float32) * 2.0**-6).astype(e5)

    # [256, 4096] -> [128p, 32k, 256e] -> [WSPLIT, 128, KPW, ...]
    def wlayout(a):
        return np.ascontiguousarray(a.reshape(E, KCH, 128).transpose(2, 1, 0))

    whi_l = wlayout(whi)
    whi_l = np.ascontiguousarray(
        whi_l.reshape(128, WSPLIT, KCH // WSPLIT, E).transpose(1, 0, 2, 3))
    wdr_l = np.stack([wlayout(wl8), wlayout(wh8)], axis=2)  # [128, 32, 2, 256]
    wdr_l = np.ascontiguousarray(
        wdr_l.reshape(128, WSPLIT, KCH // WSPLIT, 2, E).transpose(1, 0, 2, 3, 4))
    btab = np.ascontiguousarray(np.broadcast_to(expert_bias.astype(np.float32), (128, E)))

    in_maps = []
    for c in range(NCORES):
        xs = x[c * TPC:(c + 1) * TPC]
        xhi = xs.astype(np.float16)
        xlo = xs - xhi.astype(np.float32)
        # slot0 = xhi*2^-7, slot1 = xlo*2^6 (partners of wlo*2^7 / whi*2^-6)
        xh8 = (xhi.astype(np.float32) * 2.0**-7).astype(e5)
        xl8 = (xlo * 2.0**6).astype(e5)

        # [2048, 4096] -> [16i, 128p(h), 32k, 128t]
        def xlayout(a):
            return np.ascontiguousarray(
                a.reshape(NTILES, 128, KCH, 128).transpose(0, 3, 2, 1))

        xhi_l = xlayout(xhi)
        xdr_l = np.ascontiguousarray(
            np.stack([xlayout(xh8), xlayout(xl8)], axis=3))  # [16, 128, 32, 2, 128]
        in_maps.append({"xhi": xhi_l, "xdr": xdr_l, "whi": whi_l, "wdr": wdr_l, "btab": btab})
    return in_maps


def kernel(hidden_states, weight, expert_bias, _trace=False):
    from concourse.bass_utils import run_bass_kernel_spmd

    if "nc" not in _cache:
        _cache["nc"] = _build()
    nc = _cache["nc"]
    in_maps = _prep(hidden_states, weight, expert_bias)
    res = run_bass_kernel_spmd(nc, in_maps, core_ids=list(range(NCORES)), trace=_trace)
    _cache["last_results"] = res
    w = np.concatenate([res.results[c]["w_out"] for c in range(NCORES)], axis=0)
    idx = np.concatenate([res.results[c]["i_out"] for c in range(NCORES)], axis=0)
    return w.astype(np.float32), idx.astype(np.int32)


# revision 23
# speedup vs baseline: 1.0995x; 1.0995x over previous
"""MoE gate (LLaDA2) routing kernel for 8 Trainium2 NeuronCores.

Strategy: token-parallel over 8 cores (2048 tokens/core). Router GEMM as
fp16 main term (xhi@whi, exact at FP22 internal precision) plus BOTH fp16
residual cross-terms (xhi@wlo + xlo@whi) computed in a single fp8-e5m2
DoubleRow matmul per contraction chunk. The e5m2 operands carry compensating
power-of-two scales (xhi*2^-7 (x) wlo*2^7, xlo*2^6 (x) whi*2^-6) so the DR
products are unscaled and accumulate straight into the main fp32 PSUM —
2 matmuls/chunk instead of 3, ~1.4x less PE streaming than the 3-term split,
with routing accuracy within a few flipped indices of the fp32 reference.
Routing epilogue: fused scalar_tensor_tensor selection + gpsimd offload.
"""
import sys
for p in ("/opt/trn_rl_repo", "/root/.axon_site/_ro/trn_rl_repo"):
    if p not in sys.path:
        sys.path.append(p)

import numpy as np

T, H, E = 16384, 4096, 256
NCORES = 8
TPC = T // NCORES          # tokens per core: 2048
NTILES = TPC // 128        # 16 row tiles
KCH = H // 128             # 32 contraction chunks
WSPLIT = 16                # w DMA split for early start
G = 8                      # expert groups
GS = E // G                # 32 experts/group
K = 8                      # top-k
NEG = -1.0e4

_cache = {}


def _build():
    import concourse.bacc as bacc
    import concourse.bass as bass
    import concourse.mybir as mybir
    from concourse import tile

    dt = mybir.dt
    Alu = mybir.AluOpType
    Act = mybir.ActivationFunctionType
    Ax = mybir.AxisListType
    DR = mybir.MatmulPerfMode.DoubleRow

    nc = bacc.Bacc("TRN2", target_bir_lowering=False, debug=False,
                   num_devices=NCORES)

    xhi_d = nc.dram_tensor("xhi", [NTILES, 128, KCH, 128], dt.float16, kind="ExternalInput")
    xdr_d = nc.dram_tensor("xdr", [NTILES, 128, KCH, 2, 128], dt.float8e5, kind="ExternalInput")
    whi_d = nc.dram_tensor("whi", [WSPLIT, 128, KCH // WSPLIT, E], dt.float16, kind="ExternalInput")
    wdr_d = nc.dram_tensor("wdr", [WSPLIT, 128, KCH // WSPLIT, 2, E], dt.float8e5, kind="ExternalInput")
    btab_d = nc.dram_tensor("btab", [128, E], dt.float32, kind="ExternalInput")
    w_out = nc.dram_tensor("w_out", [TPC, K], dt.float32, kind="ExternalOutput")
    i_out = nc.dram_tensor("i_out", [TPC, K], dt.uint32, kind="ExternalOutput")

    KPW = KCH // WSPLIT  # k-chunks per w split

    def bc_mid(ap8, n=8):
        # [128, m] -> [128, n(bcast), m]
        return bass.AP(ap8.tensor, ap8.offset, [list(ap8.ap[0]), [0, n], list(ap8.ap[1])])

    def pair_ap(t, elem_off, m):
        # slice of flat SBUF tile -> [128, 2, m] AP (pair-slot stride m)
        a = t[:]
        return bass.AP(a.tensor, a.offset + elem_off, [list(a.ap[0]), [m, 2], [1, m]])

    with tile.TileContext(nc) as tc:
        with (
            tc.tile_pool(name="wpool", bufs=1) as wpool,
            tc.tile_pool(name="xpool", bufs=4) as xpool,
            tc.tile_pool(name="ppool", bufs=4, space="PSUM") as ppool,
            tc.tile_pool(name="spool", bufs=3) as spool,
            tc.tile_pool(name="tpool", bufs=3) as tpool,
            tc.tile_pool(name="opool", bufs=1) as opool,
        ):
            whis, wdrs = [], []
            for s in range(WSPLIT):
                whi_t = wpool.tile([128, KPW * E], dt.float16, tag=f"whi{s}")
                wdr_t = wpool.tile([128, KPW * 2 * E], dt.float8e5, tag=f"wdr{s}")
                whis.append(whi_t)
                wdrs.append(wdr_t)
            nc.sync.dma_start(whis[0][:], whi_d[0].rearrange("p k e -> p (k e)"))
            nc.sync.dma_start(wdrs[0][:], wdr_d[0].rearrange("p k u e -> p (k u e)"))
            btab = wpool.tile([128, E], dt.float32, tag="btab")
            nc.sync.dma_start(btab[:], btab_d[:])

            out_w = opool.tile([128, NTILES * K], dt.float32, tag="ow")
            out_i = opool.tile([128, NTILES * K], dt.uint32, tag="oi")

            XG = 4                    # x k-group split for DMA granularity
            KPX = KCH // XG
            for i in range(NTILES):
                xhgs, xdgs = [], []
                for g in range(XG):
                    xhg = xpool.tile([128, KPX * 128], dt.float16, tag=f"xh{g}")
                    nc.sync.dma_start(xhg[:], xhi_d[i, :, g * KPX:(g + 1) * KPX, :]
                                      .rearrange("p k t -> p (k t)"))
                    xdg = xpool.tile([128, KPX * 2 * 128], dt.float8e5, tag=f"xd{g}")
                    nc.sync.dma_start(xdg[:], xdr_d[i, :, g * KPX:(g + 1) * KPX, :, :]
                                      .rearrange("p k u t -> p (k u t)"))
                    xhgs.append(xhg)
                    xdgs.append(xdg)

                if i in (0, 1):
                    for s in ((1, 2) if i == 0 else (3,)):
                        nc.sync.dma_start(whis[s][:], whi_d[s].rearrange("p k e -> p (k e)"))
                        nc.sync.dma_start(wdrs[s][:], wdr_d[s].rearrange("p k u e -> p (k u e)"))

                psum = ppool.tile([128, E], dt.float32, tag="ps")
                for k in range(KCH):
                    xh = xhgs[k // KPX][:, (k % KPX) * 128:(k % KPX + 1) * 128]
                    wh = whis[k // KPW][:, (k % KPW) * E:(k % KPW + 1) * E]
                    nc.tensor.matmul(psum[:], lhsT=xh, rhs=wh,
                                     start=(k == 0), stop=False)
                    xd3 = pair_ap(xdgs[k // KPX], (k % KPX) * 2 * 128, 128)
                    wd3 = pair_ap(wdrs[k // KPW], (k % KPW) * 2 * E, E)
                    nc.tensor.matmul(psum[:], lhsT=xd3, rhs=wd3,
                                     start=False, stop=(k == KCH - 1), perf_mode=DR)

                # --- routing epilogue (DVE-centric, gpsimd offload) ---
                scores = spool.tile([128, E], dt.float32, tag="scores")
                nc.scalar.activation(scores[:], psum[:], Act.Sigmoid)

                # sr = scores + bias (selection scores)
                sr = spool.tile([128, E], dt.float32, tag="sr")
                nc.vector.tensor_tensor(sr[:], scores[:], btab[:], Alu.add)
                sr3 = sr[:].rearrange("p (g e) -> p g e", g=G)

                # group top-2 sum via top1 / match_replace / top2
                top1 = tpool.tile([128, G], dt.float32, tag="top1")
                nc.vector.tensor_reduce(top1[:], sr3, axis=Ax.X, op=Alu.max)
                mr2 = spool.tile([128, E], dt.float32, tag="mr2")
                nc.vector.match_replace(mr2[:], in_to_replace=top1[:], in_values=sr[:], imm_value=NEG)
                top2 = tpool.tile([128, G], dt.float32, tag="top2")
                nc.vector.tensor_reduce(top2[:], mr2[:].rearrange("p (g e) -> p g e", g=G),
                                        axis=Ax.X, op=Alu.max)
                gs_t = tpool.tile([128, G], dt.float32, tag="gs")
                nc.vector.tensor_tensor(gs_t[:], top1[:], top2[:], Alu.add)

                # keep top-4 groups: threshold at 4th largest group score
                g8 = tpool.tile([128, 8], dt.float32, tag="g8")
                nc.vector.max(out=g8[:], in_=gs_t[:])
                inv = tpool.tile([128, G], dt.float32, tag="inv")
                nc.vector.tensor_scalar(inv[:], gs_t[:], g8[:, 3:4], -NEG, op0=Alu.is_lt, op1=Alu.mult)
                # mask: sr -= inv (0 for kept groups, 1e4 for dropped)
                nc.vector.tensor_tensor(sr3, sr3, inv[:].to_broadcast([128, G, GS]), Alu.subtract)

                # top-8 selection on masked sr
                vals8 = tpool.tile([128, K], dt.float32, tag="vals8")
                nc.vector.max(out=vals8[:], in_=sr[:])
                idx8 = tpool.tile([128, K], dt.uint32, tag="idx8")
                nc.vector.max_index(out=idx8[:], in_max=vals8[:], in_values=sr[:])

                # scores at selected positions: (sr >= t8) * scores, one fused op
                selm = spool.tile([128, E], dt.float32, tag="selm")
                nc.vector.scalar_tensor_tensor(selm[:], sr[:], vals8[:, 7:8], scores[:],
                                               op0=Alu.is_ge, op1=Alu.mult)
                svals8 = tpool.tile([128, K], dt.float32, tag="svals8")
                nc.vector.max(out=svals8[:], in_=selm[:])
                sidx8 = tpool.tile([128, K], dt.uint32, tag="sidx8")
                nc.vector.max_index(out=sidx8[:], in_max=svals8[:], in_values=selm[:])

                # reorder svals8 (score-sorted) into idx8 (sr-sorted) slots
                idx8f = tpool.tile([128, K], dt.float32, tag="idx8f")
                nc.gpsimd.tensor_copy(idx8f[:], idx8[:])
                sidx8f = tpool.tile([128, K], dt.float32, tag="sidx8f")
                nc.gpsimd.tensor_copy(sidx8f[:], sidx8[:])
                eq = tpool.tile([128, K * K], dt.float32, tag="eq")
                eq3 = eq[:].rearrange("p (k j) -> p k j", k=K)
                nc.vector.tensor_tensor(eq3, idx8f[:].to_broadcast([128, K, K]), bc_mid(sidx8f[:]), Alu.is_equal)
                prod = tpool.tile([128, K * K], dt.float32, tag="prod")
                prod3 = prod[:].rearrange("p (k j) -> p k j", k=K)
                nc.vector.tensor_tensor(prod3, eq3, bc_mid(svals8[:]), Alu.mult)
                w8 = tpool.tile([128, K], dt.float32, tag="w8")
                nc.vector.tensor_reduce(w8[:], prod3, axis=Ax.X, op=Alu.add)

                sum8 = tpool.tile([128, 1], dt.float32, tag="sum8")
                nc.vector.tensor_reduce(sum8[:], w8[:], axis=Ax.X, op=Alu.add)
                rec = tpool.tile([128, 1], dt.float32, tag="rec")
                nc.vector.reciprocal(rec[:], sum8[:])

                nc.vector.tensor_scalar(out_w[:, i * K:(i + 1) * K], w8[:], rec[:, 0:1], 2.5,
                                        op0=Alu.mult, op1=Alu.mult)
                nc.gpsimd.tensor_copy(out_i[:, i * K:(i + 1) * K], idx8[:])

            nc.sync.dma_start(w_out[:].rearrange("(i p) k -> p i k", p=128),
                              out_w[:].rearrange("p (i k) -> p i k", i=NTILES))
            nc.sync.dma_start(i_out[:].rearrange("(i p) k -> p i k", p=128),
                              out_i[:].rearrange("p (i k) -> p i k", i=NTILES))

    nc.compile()
    return nc


def _prep(hidden_states, weight, expert_bias):
    import ml_dtypes
    e5 = ml_dtypes.float8_e5m2
    x = np.ascontiguousarray(hidden_states, dtype=np.float32)
    w = np.ascontiguousarray(weight, dtype=np.float32)
    whi = w.astype(np.float16)
    wlo = w - whi.astype(np.float32)
    # DR pair slots (scale-compensated e5m2): slot0 = wlo*2^7, slot1 = whi*2^-6
    wl8 = (wlo * 2.0**7).astype(e5)
    wh8 = (whi.astype(np.float32) * 2.0**-6).astype(e5)

    # [256, 4096] -> [128p, 32k, 256e] -> [WSPLIT, 128, KPW, ...]
    def wlayout(a):
        return np.ascontiguousarray(a.reshape(E, KCH, 128).transpose(2, 1, 0))

    whi_l = wlayout(whi)
    whi_l = np.ascontiguousarray(
        whi_l.reshape(128, WSPLIT, KCH // WSPLIT, E).transpose(1, 0, 2, 3))
    wdr_l = np.stack([wlayout(wl8), wlayout(wh8)], axis=2)  # [128, 32, 2, 256]
    wdr_l = np.ascontiguousarray(
        wdr_l.reshape(128, WSPLIT, KCH // WSPLIT, 2, E).transpose(1, 0, 2, 3, 4))
    btab = np.ascontiguousarray(np.broadcast_to(expert_bias.astype(np.float32), (128, E)))

    in_maps = []
    for c in range(NCORES):
        xs = x[c * TPC:(c + 1) * TPC]
        xhi = xs.astype(np.float16)
        xlo = xs - xhi.astype(np.float32)
        # slot0 = xhi*2^-7, slot1 = xlo*2^6 (partners of wlo*2^7 / whi*2^-6)
        xh8 = (xhi.astype(np.float32) * 2.0**-7).astype(e5)
        xl8 = (xlo * 2.0**6).astype(e5)

        # [2048, 4096] -> [16i, 128p(h), 32k, 128t]
        def xlayout(a):
            return np.ascontiguousarray(
                a.reshape(NTILES, 128, KCH, 128).transpose(0, 3, 2, 1))

        xhi_l = xlayout(xhi)
        xdr_l = np.ascontiguousarray(
            np.stack([xlayout(xh8), xlayout(xl8)], axis=3))  # [16, 128, 32, 2, 128]
        in_maps.append({"xhi": xhi_l, "xdr": xdr_l, "whi": whi_l, "wdr": wdr_l, "btab": btab})
    return in_maps


def kernel(hidden_states, weight, expert_bias, _trace=False):
    from concourse.bass_utils import run_bass_kernel_spmd

    if "nc" not in _cache:
        _cache["nc"] = _build()
    nc = _cache["nc"]
    in_maps = _prep(hidden_states, weight, expert_bias)
    res = run_bass_kernel_spmd(nc, in_maps, core_ids=list(range(NCORES)), trace=_trace)
    _cache["last_results"] = res
    w = np.concatenate([res.results[c]["w_out"] for c in range(NCORES)], axis=0)
    idx = np.concatenate([res.results[c]["i_out"] for c in range(NCORES)], axis=0)
    return w.astype(np.float32), idx.astype(np.int32)


# revision 25
# speedup vs baseline: 1.1602x; 1.0551x over previous
"""MoE gate (LLaDA2) routing kernel for 8 Trainium2 NeuronCores.

Strategy: token-parallel over 8 cores (2048 tokens/core). Router GEMM as
fp16 main term (xhi@whi, exact at FP22 internal precision) plus BOTH fp16
residual cross-terms (xhi@wlo + xlo@whi) computed in a single fp8-e5m2
DoubleRow matmul per contraction chunk. The e5m2 operands carry compensating
power-of-two scales (xhi*2^-7 (x) wlo*2^7, xlo*2^6 (x) whi*2^-6) so the DR
products are unscaled and accumulate straight into the main fp32 PSUM —
2 matmuls/chunk instead of 3, ~1.4x less PE streaming than the 3-term split,
with routing accuracy within a few flipped indices of the fp32 reference.
Routing epilogue: fused scalar_tensor_tensor selection + gpsimd offload.
"""
import sys
for p in ("/opt/trn_rl_repo", "/root/.axon_site/_ro/trn_rl_repo"):
    if p not in sys.path:
        sys.path.append(p)

import numpy as np

T, H, E = 16384, 4096, 256
NCORES = 8
TPC = T // NCORES          # tokens per core: 2048
NTILES = TPC // 128        # 16 row tiles
KCH = H // 128             # 32 contraction chunks
WSPLIT = 8                 # w DMA split for early start
G = 8                      # expert groups
GS = E // G                # 32 experts/group
K = 8                      # top-k
NEG = -1.0e4

_cache = {}


def _build():
    import concourse.bacc as bacc
    import concourse.bass as bass
    import concourse.mybir as mybir
    from concourse import tile

    dt = mybir.dt
    Alu = mybir.AluOpType
    Act = mybir.ActivationFunctionType
    Ax = mybir.AxisListType
    DR = mybir.MatmulPerfMode.DoubleRow

    nc = bacc.Bacc("TRN2", target_bir_lowering=False, debug=False,
                   num_devices=NCORES)

    xhi_d = nc.dram_tensor("xhi", [NTILES, 128, KCH, 128], dt.float16, kind="ExternalInput")
    xdr_d = nc.dram_tensor("xdr", [NTILES, 128, KCH, 2, 128], dt.float8e5, kind="ExternalInput")
    whi_d = nc.dram_tensor("whi", [WSPLIT, 128, KCH // WSPLIT, E], dt.float16, kind="ExternalInput")
    wdr_d = nc.dram_tensor("wdr", [WSPLIT, 128, KCH // WSPLIT, 2, E], dt.float8e5, kind="ExternalInput")
    btab_d = nc.dram_tensor("btab", [128, E], dt.float32, kind="ExternalInput")
    w_out = nc.dram_tensor("w_out", [TPC, K], dt.float32, kind="ExternalOutput")
    i_out = nc.dram_tensor("i_out", [TPC, K], dt.uint32, kind="ExternalOutput")

    KPW = KCH // WSPLIT  # k-chunks per w split

    def bc_mid(ap8, n=8):
        # [128, m] -> [128, n(bcast), m]
        return bass.AP(ap8.tensor, ap8.offset, [list(ap8.ap[0]), [0, n], list(ap8.ap[1])])

    def pair_ap(t, elem_off, m):
        # slice of flat SBUF tile -> [128, 2, m] AP (pair-slot stride m)
        a = t[:]
        return bass.AP(a.tensor, a.offset + elem_off, [list(a.ap[0]), [m, 2], [1, m]])

    with tile.TileContext(nc) as tc:
        with (
            tc.tile_pool(name="wpool", bufs=1) as wpool,
            tc.tile_pool(name="xpool", bufs=4) as xpool,
            tc.tile_pool(name="ppool", bufs=4, space="PSUM") as ppool,
            tc.tile_pool(name="spool", bufs=4) as spool,
            tc.tile_pool(name="tpool", bufs=4) as tpool,
            tc.tile_pool(name="opool", bufs=1) as opool,
        ):
            whis, wdrs = [], []
            for s in range(WSPLIT):
                whi_t = wpool.tile([128, KPW * E], dt.float16, tag=f"whi{s}")
                wdr_t = wpool.tile([128, KPW * 2 * E], dt.float8e5, tag=f"wdr{s}")
                whis.append(whi_t)
                wdrs.append(wdr_t)
            nc.sync.dma_start(whis[0][:], whi_d[0].rearrange("p k e -> p (k e)"))
            nc.sync.dma_start(wdrs[0][:], wdr_d[0].rearrange("p k u e -> p (k u e)"))
            btab = wpool.tile([128, E], dt.float32, tag="btab")
            nc.sync.dma_start(btab[:], btab_d[:])

            out_w = opool.tile([128, NTILES * K], dt.float32, tag="ow")
            out_i = opool.tile([128, NTILES * K], dt.uint32, tag="oi")

            XG = 4                    # x k-group split for DMA granularity
            KPX = KCH // XG
            for i in range(NTILES):
                xhgs, xdgs = [], []
                for g in range(XG):
                    xhg = xpool.tile([128, KPX * 128], dt.float16, tag=f"xh{g}")
                    nc.sync.dma_start(xhg[:], xhi_d[i, :, g * KPX:(g + 1) * KPX, :]
                                      .rearrange("p k t -> p (k t)"))
                    xdg = xpool.tile([128, KPX * 2 * 128], dt.float8e5, tag=f"xd{g}")
                    nc.sync.dma_start(xdg[:], xdr_d[i, :, g * KPX:(g + 1) * KPX, :, :]
                                      .rearrange("p k u t -> p (k u t)"))
                    xhgs.append(xhg)
                    xdgs.append(xdg)

                if i in (0, 1):
                    for s in ((1, 2) if i == 0 else (3,)):
                        nc.sync.dma_start(whis[s][:], whi_d[s].rearrange("p k e -> p (k e)"))
                        nc.sync.dma_start(wdrs[s][:], wdr_d[s].rearrange("p k u e -> p (k u e)"))

                psum = ppool.tile([128, E], dt.float32, tag="ps")
                for k in range(KCH):
                    xh = xhgs[k // KPX][:, (k % KPX) * 128:(k % KPX + 1) * 128]
                    wh = whis[k // KPW][:, (k % KPW) * E:(k % KPW + 1) * E]
                    nc.tensor.matmul(psum[:], lhsT=xh, rhs=wh,
                                     start=(k == 0), stop=False)
                    xd3 = pair_ap(xdgs[k // KPX], (k % KPX) * 2 * 128, 128)
                    wd3 = pair_ap(wdrs[k // KPW], (k % KPW) * 2 * E, E)
                    nc.tensor.matmul(psum[:], lhsT=xd3, rhs=wd3,
                                     start=False, stop=(k == KCH - 1), perf_mode=DR)

                # --- routing epilogue (DVE-centric, gpsimd offload) ---
                scores = spool.tile([128, E], dt.float32, tag="scores")
                nc.scalar.activation(scores[:], psum[:], Act.Sigmoid)

                # sr = scores + bias (selection scores)
                sr = spool.tile([128, E], dt.float32, tag="sr")
                nc.vector.tensor_tensor(sr[:], scores[:], btab[:], Alu.add)
                sr3 = sr[:].rearrange("p (g e) -> p g e", g=G)

                # group top-2 sum via top1 / match_replace / top2
                top1 = tpool.tile([128, G], dt.float32, tag="top1")
                nc.vector.tensor_reduce(top1[:], sr3, axis=Ax.X, op=Alu.max)
                mr2 = spool.tile([128, E], dt.float32, tag="mr2")
                nc.vector.match_replace(mr2[:], in_to_replace=top1[:], in_values=sr[:], imm_value=NEG)
                top2 = tpool.tile([128, G], dt.float32, tag="top2")
                nc.vector.tensor_reduce(top2[:], mr2[:].rearrange("p (g e) -> p g e", g=G),
                                        axis=Ax.X, op=Alu.max)
                gs_t = tpool.tile([128, G], dt.float32, tag="gs")
                nc.vector.tensor_tensor(gs_t[:], top1[:], top2[:], Alu.add)

                # keep top-4 groups: threshold at 4th largest group score
                g8 = tpool.tile([128, 8], dt.float32, tag="g8")
                nc.vector.max(out=g8[:], in_=gs_t[:])
                inv = tpool.tile([128, G], dt.float32, tag="inv")
                nc.vector.tensor_scalar(inv[:], gs_t[:], g8[:, 3:4], -NEG, op0=Alu.is_lt, op1=Alu.mult)
                # mask: sr -= inv (0 for kept groups, 1e4 for dropped)
                nc.vector.tensor_tensor(sr3, sr3, inv[:].to_broadcast([128, G, GS]), Alu.subtract)

                # top-8 selection on masked sr
                vals8 = tpool.tile([128, K], dt.float32, tag="vals8")
                nc.vector.max(out=vals8[:], in_=sr[:])
                idx8 = tpool.tile([128, K], dt.uint32, tag="idx8")
                nc.vector.max_index(out=idx8[:], in_max=vals8[:], in_values=sr[:])

                # scores at selected positions: (sr >= t8) * scores, one fused op
                selm = spool.tile([128, E], dt.float32, tag="selm")
                nc.vector.scalar_tensor_tensor(selm[:], sr[:], vals8[:, 7:8], scores[:],
                                               op0=Alu.is_ge, op1=Alu.mult)
                svals8 = tpool.tile([128, K], dt.float32, tag="svals8")
                nc.vector.max(out=svals8[:], in_=selm[:])
                sidx8 = tpool.tile([128, K], dt.uint32, tag="sidx8")
                nc.vector.max_index(out=sidx8[:], in_max=svals8[:], in_values=selm[:])

                # reorder svals8 (score-sorted) into idx8 (sr-sorted) slots
                idx8f = tpool.tile([128, K], dt.float32, tag="idx8f")
                nc.gpsimd.tensor_copy(idx8f[:], idx8[:])
                sidx8f = tpool.tile([128, K], dt.float32, tag="sidx8f")
                nc.gpsimd.tensor_copy(sidx8f[:], sidx8[:])
                eq = tpool.tile([128, K * K], dt.float32, tag="eq")
                eq3 = eq[:].rearrange("p (k j) -> p k j", k=K)
                nc.vector.tensor_tensor(eq3, idx8f[:].to_broadcast([128, K, K]), bc_mid(sidx8f[:]), Alu.is_equal)
                prod = tpool.tile([128, K * K], dt.float32, tag="prod")
                prod3 = prod[:].rearrange("p (k j) -> p k j", k=K)
                nc.vector.tensor_tensor(prod3, eq3, bc_mid(svals8[:]), Alu.mult)
                w8 = tpool.tile([128, K], dt.float32, tag="w8")
                nc.vector.tensor_reduce(w8[:], prod3, axis=Ax.X, op=Alu.add)

                sum8 = tpool.tile([128, 1], dt.float32, tag="sum8")
                nc.vector.tensor_reduce(sum8[:], w8[:], axis=Ax.X, op=Alu.add)
                rec = tpool.tile([128, 1], dt.float32, tag="rec")
                nc.vector.reciprocal(rec[:], sum8[:])

                nc.vector.tensor_scalar(out_w[:, i * K:(i + 1) * K], w8[:], rec[:, 0:1], 2.5,
                                        op0=Alu.mult, op1=Alu.mult)
                nc.gpsimd.tensor_copy(out_i[:, i * K:(i + 1) * K], idx8[:])

            nc.sync.dma_start(w_out[:].rearrange("(i p) k -> p i k", p=128),
                              out_w[:].rearrange("p (i k) -> p i k", i=NTILES))
            nc.sync.dma_start(i_out[:].rearrange("(i p) k -> p i k", p=128),
                              out_i[:].rearrange("p (i k) -> p i k", i=NTILES))

    nc.compile()
    return nc


def _prep(hidden_states, weight, expert_bias):
    import ml_dtypes
    e5 = ml_dtypes.float8_e5m2
    x = np.ascontiguousarray(hidden_states, dtype=np.float32)
    w = np.ascontiguousarray(weight, dtype=np.float32)
    whi = w.astype(np.float16)
    wlo = w - whi.astype(np.float32)
    # DR pair slots (scale-compensated e5m2): slot0 = wlo*2^7, slot1 = whi*2^-6
    wl8 = (wlo * 2.0**7).astype(e5)
    wh8 = (whi.astype(np.float32) * 2.0**-6).astype(e5)

    # [256, 4096] -> [128p, 32k, 256e] -> [WSPLIT, 128, KPW, ...]
    def wlayout(a):
        return np.ascontiguousarray(a.reshape(E, KCH, 128).transpose(2, 1, 0))

    whi_l = wlayout(whi)
    whi_l = np.ascontiguousarray(
        whi_l.reshape(128, WSPLIT, KCH // WSPLIT, E).transpose(1, 0, 2, 3))
    wdr_l = np.stack([wlayout(wl8), wlayout(wh8)], axis=2)  # [128, 32, 2, 256]
    wdr_l = np.ascontiguousarray(
        wdr_l.reshape(128, WSPLIT, KCH // WSPLIT, 2, E).transpose(1, 0, 2, 3, 4))
    btab = np.ascontiguousarray(np.broadcast_to(expert_bias.astype(np.float32), (128, E)))

    in_maps = []
    for c in range(NCORES):
        xs = x[c * TPC:(c + 1) * TPC]
        xhi = xs.astype(np.float16)
        xlo = xs - xhi.astype(np.float32)
        # slot0 = xhi*2^-7, slot1 = xlo*2^6 (partners of wlo*2^7 / whi*2^-6)
        xh8 = (xhi.astype(np.float32) * 2.0**-7).astype(e5)
        xl8 = (xlo * 2.0**6).astype(e5)

        # [2048, 4096] -> [16i, 128p(h), 32k, 128t]
        def xlayout(a):
            return np.ascontiguousarray(
                a.reshape(NTILES, 128, KCH, 128).transpose(0, 3, 2, 1))

        xhi_l = xlayout(xhi)
        xdr_l = np.ascontiguousarray(
            np.stack([xlayout(xh8), xlayout(xl8)], axis=3))  # [16, 128, 32, 2, 128]
        in_maps.append({"xhi": xhi_l, "xdr": xdr_l, "whi": whi_l, "wdr": wdr_l, "btab": btab})
    return in_maps


def kernel(hidden_states, weight, expert_bias, _trace=False):
    from concourse.bass_utils import run_bass_kernel_spmd

    if "nc" not in _cache:
        _cache["nc"] = _build()
    nc = _cache["nc"]
    in_maps = _prep(hidden_states, weight, expert_bias)
    res = run_bass_kernel_spmd(nc, in_maps, core_ids=list(range(NCORES)), trace=_trace)
    _cache["last_results"] = res
    w = np.concatenate([res.results[c]["w_out"] for c in range(NCORES)], axis=0)
    idx = np.concatenate([res.results[c]["i_out"] for c in range(NCORES)], axis=0)
    return w.astype(np.float32), idx.astype(np.int32)
